# revision 20
# baseline (speedup 1.0000x reference)
"""AttnGRU Trainium2 kernel: 8-way data-parallel, H-major dataflow.

Math per core (B_loc=32, T=128, H=1024):
  xr = x @ Wr_w.T + (Wr_b + Ur_b)      (precomputed, blocked over time)
  xn = x @ W_w.T  + W_b
  per step: rt = sigmoid(xr_t + h @ Ur_w.T)
            nt = tanh(xn_t + rt * (h @ U_w.T + U_b))
            h  = (1-gt)*nt + gt*h

Everything on-device lives H-major ("transposed"): tiles are
[128 partitions = H-chunk, free = (chunk, batch)] so elementwise ops use
all 128 partitions and the recurrent matmul output lands already in the
layout the next step consumes (no per-step transposes).
"""

import numpy as np
import ml_dtypes

import concourse.bass as bass
import concourse.bacc as bacc
import concourse.mybir as mybir
from concourse import tile
from concourse.tile import add_dep_helper
from concourse.bass_utils import run_bass_kernel_spmd

B, T, H = 256, 128, 1024
NCORES = 8
BL = B // NCORES          # 32 batch rows per core
BT = BL * T               # 4096 (time-major: col = t*32 + b)
KC = H // 128             # 8 contraction chunks
MC = 2048 // 128          # 16 output chunks ([r | n] concat)
BLK = 8                   # scan steps per precompute block
NBLK = T // BLK           # 16
RING = 2 * BLK            # ring of per-step slots (512 cols each)

BF = mybir.dt.bfloat16
F32 = mybir.dt.float32
AF = mybir.ActivationFunctionType
OP = mybir.AluOpType

_CACHE = {}


def _build_bass():
    nc = bacc.Bacc()
    xT = nc.declare_dram_parameter("xT", [H, BT], BF, isOutput=False)
    wpreT = nc.declare_dram_parameter("wpreT", [H, 2048], BF, isOutput=False)
    uuT = nc.declare_dram_parameter("uuT", [H, 2048], BF, isOutput=False)
    biasT = nc.declare_dram_parameter("biasT", [128, 512], BF, isOutput=False)
    wbT = nc.declare_dram_parameter("wbT", [128, 256], F32, isOutput=False)
    gtT = nc.declare_dram_parameter("gtT", [128, 256], F32, isOutput=False)
    h0T = nc.declare_dram_parameter("h0T", [128, 256], F32, isOutput=False)
    ident = nc.declare_dram_parameter("ident", [128, 128], BF, isOutput=False)
    out = nc.declare_dram_parameter("out", [128, 256], F32, isOutput=True)

    with tile.TileContext(nc) as tc:
        with (
            tc.tile_pool(name="w", bufs=1) as wp,
            tc.tile_pool(name="ew", bufs=3) as ew,
            tc.tile_pool(name="ps", bufs=1, space="PSUM") as psp,
            tc.tile_pool(name="pp", bufs=1, space="PSUM") as ppp,
        ):
            xT_sb = [wp.tile([128, BT], BF, tag=f"xT{k}", name=f"xT{k}") for k in range(KC)]
            uu_sb = [wp.tile([128, 2048], BF, tag=f"uu{k}", name=f"uu{k}") for k in range(KC)]
            wpre_sb = [wp.tile([128, 2048], BF, tag=f"wp{k}", name=f"wp{k}") for k in range(KC)]
            ring = wp.tile([128, RING * 512], BF, tag="ring")
            bias_sb = wp.tile([128, 512], BF, tag="bias")
            wb_sb = wp.tile([128, 256], F32, tag="wb")
            gt_sb = wp.tile([128, 256], F32, tag="gt")
            id_sb = wp.tile([128, 128], BF, tag="id")
            hT = wp.tile([128, 256], F32, tag="hT")
            hbf = wp.tile([128, 256], BF, tag="hbf")

            for k in range(KC):
                nc.sync.dma_start(out=uu_sb[k][:, :], in_=uuT[k * 128:(k + 1) * 128, :])
                nc.sync.dma_start(out=wpre_sb[k][:, :], in_=wpreT[k * 128:(k + 1) * 128, :])
                nc.sync.dma_start(out=xT_sb[k][:, :], in_=xT[k * 128:(k + 1) * 128, :])
            nc.sync.dma_start(out=gt_sb[:, :], in_=gtT[:, :])
            nc.sync.dma_start(out=bias_sb[:, :], in_=biasT[:, :])
            nc.sync.dma_start(out=wb_sb[:, :], in_=wbT[:, :])
            nc.sync.dma_start(out=id_sb[:, :], in_=ident[:, :])
            nc.sync.dma_start(out=hT[:, :], in_=h0T[:, :])
            nc.vector.tensor_copy(hbf[:, :], hT[:, :])

            ring3 = ring[:, :].rearrange("p (s c) -> p s c", c=512)

            NPRE = 4         # distinct precompute psum buffers

            def precompute_block(i):
                # xr/xn for steps i*BLK .. (i+1)*BLK, into ring slots (i%2)*BLK ..
                s0 = (i % 2) * BLK
                cps = []
                for m in range(MC):
                    slot = (i * MC + m) % NPRE
                    ps = ppp.tile([128, BLK * 32], F32, tag=f"pre{slot}",
                                  name=f"pre{i}_{m}", padded_shape=[128, 512])
                    for k in range(KC):
                        nc.tensor.matmul(
                            ps[:, :],
                            wpre_sb[k][:, m * 128:(m + 1) * 128],
                            xT_sb[k][:, i * BLK * 32:(i + 1) * BLK * 32],
                            start=(k == 0),
                            stop=(k == KC - 1),
                        )
                    dst = ring3[:, s0:s0 + BLK, m * 32:(m + 1) * 32]
                    src = ps[:, :].rearrange("p (s c) -> p s c", c=32)
                    nc.vector.tensor_copy(dst, src)

            def scan_step(t):
                base = (t % RING) * 512
                slot = t % 2
                ps_r = psp.tile([128, 256], F32, tag=f"pr{slot}", name=f"psr{t}",
                                padded_shape=[128, 512])
                ps_n = psp.tile([128, 256], F32, tag=f"pn{slot}", name=f"psn{t}",
                                padded_shape=[128, 512])
                # Initialize PSUM with the additive terms via identity matmuls:
                # ps_r = bias_r + xr_t, ps_n = U_b.
                nc.tensor.matmul(ps_r[:, :], id_sb[:, :], bias_sb[:, 0:256],
                                 start=True, stop=False, skip_group_check=True)
                nc.tensor.matmul(ps_n[:, :], id_sb[:, :], bias_sb[:, 256:512],
                                 start=True, stop=False, skip_group_check=True)
                nc.tensor.matmul(ps_r[:, :], id_sb[:, :], ring[:, base:base + 256],
                                 start=False, stop=False, skip_group_check=True)
                for m in range(MC):
                    half = ps_r if m < 8 else ps_n
                    col = (m % 8) * 32
                    for k in range(KC):
                        nc.tensor.matmul(
                            half[:, col:col + 32],
                            uu_sb[k][:, m * 128:(m + 1) * 128],
                            hbf[:, k * 32:(k + 1) * 32],
                            start=False,
                            stop=(k == KC - 1),
                            skip_group_check=True,
                        )
                rt = ew.tile([128, 256], F32, tag="rt", name=f"rt{t}")
                nc.scalar.activation(rt[:, :], ps_r[:, :], AF.Sigmoid)
                n2 = ew.tile([128, 256], F32, tag="n2", name=f"n2{t}")
                nc.vector.tensor_tensor(n2[:, :], rt[:, :], ps_n[:, :], OP.mult)
                a0 = ew.tile([128, 256], F32, tag="a0", name=f"a0{t}")
                nc.vector.tensor_tensor(a0[:, :], n2[:, :], wb_sb[:, :], OP.add)
                an = ew.tile([128, 256], F32, tag="an", name=f"an{t}")
                nc.vector.tensor_tensor(an[:, :], a0[:, :],
                                        ring[:, base + 256:base + 512], OP.add)
                nt = ew.tile([128, 256], F32, tag="nt", name=f"nt{t}")
                nc.scalar.activation(nt[:, :], an[:, :], AF.Tanh)
                d = ew.tile([128, 256], F32, tag="d", name=f"d{t}")
                nc.vector.tensor_tensor(d[:, :], hT[:, :], nt[:, :], OP.subtract)
                e = ew.tile([128, 256], F32, tag="e", name=f"e{t}")
                nc.vector.tensor_tensor(e[:, :], d[:, :], gt_sb[:, :], OP.mult)
                nc.vector.tensor_tensor(hT[:, :], nt[:, :], e[:, :], OP.add)
                nc.vector.tensor_copy(hbf[:, :], hT[:, :])

            precompute_block(0)
            precompute_block(1)
            for i in range(NBLK):
                for u in range(BLK):
                    scan_step(i * BLK + u)
                if i + 2 < NBLK:
                    precompute_block(i + 2)

            nc.sync.dma_start(out=out[:, :], in_=hT[:, :])

    nc.finalize()
    return nc


def _prep_inputs(x, h0, gt, Wr_w, Wr_b, Ur_w, Ur_b, W_w, W_b, U_w, U_b):
    bf = ml_dtypes.bfloat16
    wpreT = np.ascontiguousarray(
        np.concatenate([Wr_w.T, W_w.T], axis=1)).astype(bf)          # [H, 2048]
    uuT = np.ascontiguousarray(
        np.concatenate([Ur_w.T, U_w.T], axis=1)).astype(bf)          # [H, 2048]
    # bias_sb [128, 512]: cols 0:256 = r-side bias (Wr_b + Ur_b), cols
    # 256:512 = U_b; each laid out H-major ([p, c*32+b] = bias[128c+p]) and
    # folded into PSUM by the identity matmuls. W_b (the xn bias, which sits
    # outside the rt* product) is shipped separately and added on DVE.
    def hmajor_bcast(v):
        return np.ascontiguousarray(
            np.broadcast_to(v.reshape(8, 128).T[:, :, None],
                            (128, 8, 32)).reshape(128, 256))
    biasT = np.concatenate(
        [hmajor_bcast((Wr_b + Ur_b).astype(np.float32)),
         hmajor_bcast(U_b.astype(np.float32))], axis=1).astype(bf)
    wbT = hmajor_bcast(W_b.astype(np.float32)).astype(np.float32)
    ident = np.eye(128, dtype=bf)

    in_maps = []
    for c in range(NCORES):
        sl = slice(c * BL, (c + 1) * BL)
        x_loc = x[sl]                                  # [32, 128, 1024]
        xT = np.ascontiguousarray(
            x_loc.transpose(2, 1, 0).reshape(H, BT)).astype(bf)
        h0T = np.ascontiguousarray(
            h0[sl].reshape(BL, 8, 128).transpose(2, 1, 0).reshape(128, 256)
        ).astype(np.float32)
        gtT = np.ascontiguousarray(
            np.broadcast_to(gt[sl].reshape(BL)[None, None, :],
                            (128, 8, 32)).reshape(128, 256)).astype(np.float32)
        in_maps.append({
            "xT": xT, "wpreT": wpreT, "uuT": uuT, "biasT": biasT,
            "wbT": wbT, "gtT": gtT, "h0T": h0T, "ident": ident,
        })
    return in_maps


def kernel(x, h0, gt, Wr_w, Wr_b, Ur_w, Ur_b, Wz_w, Wz_b, Uz_w, Uz_b,
           W_w, W_b, U_w, U_b, _trace=False, _tmpdir=None):
    x = np.asarray(x, np.float32)
    h0 = np.asarray(h0, np.float32)
    gt = np.asarray(gt, np.float32)
    in_maps = _prep_inputs(x, h0, gt,
                           np.asarray(Wr_w, np.float32), np.asarray(Wr_b, np.float32),
                           np.asarray(Ur_w, np.float32), np.asarray(Ur_b, np.float32),
                           np.asarray(W_w, np.float32), np.asarray(W_b, np.float32),
                           np.asarray(U_w, np.float32), np.asarray(U_b, np.float32))
    if "nc" not in _CACHE:
        _CACHE["nc"] = _build_bass()
    res = run_bass_kernel_spmd(_CACHE["nc"], in_maps, core_ids=list(range(NCORES)),
                               trace=_trace, tmpdir=_tmpdir)
    outs = []
    for c in range(NCORES):
        o = np.asarray(res.results[c]["out"], np.float32)       # [128, 256]
        outs.append(o.reshape(128, 8, BL).transpose(2, 1, 0).reshape(BL, H))
    full = np.concatenate(outs, axis=0)                          # [256, 1024]
    if _trace:
        return full, res
    return full


# revision 25
# speedup vs baseline: 99.5246x; 99.5246x over previous
"""AttnGRU Trainium2 kernel: 8-way data-parallel, H-major dataflow.

Math per core (B_loc=32, T=128, H=1024):
  xr = x @ Wr_w.T + (Wr_b + Ur_b)      (precomputed, blocked over time)
  xn = x @ W_w.T  + W_b
  per step: rt = sigmoid(xr_t + h @ Ur_w.T)
            nt = tanh(xn_t + rt * (h @ U_w.T + U_b))
            h  = (1-gt)*nt + gt*h

Everything on-device lives H-major ("transposed"): tiles are
[128 partitions = H-chunk, free = (chunk, batch)] so elementwise ops use
all 128 partitions and the recurrent matmul output lands already in the
layout the next step consumes (no per-step transposes).
"""

import numpy as np
import ml_dtypes

import concourse.bass as bass
import concourse.bacc as bacc
import concourse.mybir as mybir
from concourse import tile
from concourse.tile import add_dep_helper
from concourse.bass_utils import run_bass_kernel_spmd

B, T, H = 256, 128, 1024
NCORES = 8
BL = B // NCORES          # 32 batch rows per core
BT = BL * T               # 4096 (time-major: col = t*32 + b)
KC = H // 128             # 8 contraction chunks
MC = 2048 // 128          # 16 output chunks ([r | n] concat)
BLK = 8                   # scan steps per precompute block
NBLK = T // BLK           # 16
RING = 2 * BLK            # ring of per-step slots (512 cols each)

BF = mybir.dt.bfloat16
F32 = mybir.dt.float32
AF = mybir.ActivationFunctionType
OP = mybir.AluOpType

_CACHE = {}


def _build_bass():
    nc = bacc.Bacc()
    xT = nc.declare_dram_parameter("xT", [H, BT], BF, isOutput=False)
    wpreT = nc.declare_dram_parameter("wpreT", [H, 2048], BF, isOutput=False)
    uuT = nc.declare_dram_parameter("uuT", [H, 2048], BF, isOutput=False)
    biasp = nc.declare_dram_parameter("biasp", [128, MC], F32, isOutput=False)
    ubT = nc.declare_dram_parameter("ubT", [128, 256], BF, isOutput=False)
    gtT = nc.declare_dram_parameter("gtT", [128, 256], F32, isOutput=False)
    h0T = nc.declare_dram_parameter("h0T", [128, 256], F32, isOutput=False)
    ident = nc.declare_dram_parameter("ident", [128, 128], BF, isOutput=False)
    out = nc.declare_dram_parameter("out", [128, 256], F32, isOutput=True)

    with tile.TileContext(nc) as tc:
        with (
            tc.tile_pool(name="w", bufs=1) as wp,
            tc.tile_pool(name="ew", bufs=3) as ew,
            tc.tile_pool(name="ps", bufs=1, space="PSUM") as psp,
            tc.tile_pool(name="pp", bufs=1, space="PSUM") as ppp,
        ):
            xT_sb = [wp.tile([128, BT], BF, tag=f"xT{k}", name=f"xT{k}") for k in range(KC)]
            uu_sb = [wp.tile([128, 2048], BF, tag=f"uu{k}", name=f"uu{k}") for k in range(KC)]
            wpre_sb = [wp.tile([128, 2048], BF, tag=f"wp{k}", name=f"wp{k}") for k in range(KC)]
            ring = wp.tile([128, RING * 512], BF, tag="ring")
            bias_sb = wp.tile([128, MC], F32, tag="bias")
            ub_sb = wp.tile([128, 256], BF, tag="ub")
            gt_sb = wp.tile([128, 256], F32, tag="gt")
            id_sb = wp.tile([128, 128], BF, tag="id")
            hT = wp.tile([128, 256], F32, tag="hT")
            hbf = wp.tile([128, 256], BF, tag="hbf")

            for k in range(KC):
                nc.sync.dma_start(out=uu_sb[k][:, :], in_=uuT[k * 128:(k + 1) * 128, :])
                nc.sync.dma_start(out=wpre_sb[k][:, :], in_=wpreT[k * 128:(k + 1) * 128, :])
                nc.sync.dma_start(out=xT_sb[k][:, :], in_=xT[k * 128:(k + 1) * 128, :])
            nc.sync.dma_start(out=gt_sb[:, :], in_=gtT[:, :])
            nc.sync.dma_start(out=bias_sb[:, :], in_=biasp[:, :])
            nc.sync.dma_start(out=ub_sb[:, :], in_=ubT[:, :])
            nc.sync.dma_start(out=id_sb[:, :], in_=ident[:, :])
            nc.sync.dma_start(out=hT[:, :], in_=h0T[:, :])
            nc.vector.tensor_copy(hbf[:, :], hT[:, :])

            ring3 = ring[:, :].rearrange("p (s c) -> p s c", c=512)

            NPRE = 4         # distinct precompute psum buffers

            def precompute_block(i):
                # xr/xn for steps i*BLK .. (i+1)*BLK, into ring slots (i%2)*BLK ..
                s0 = (i % 2) * BLK
                cps = []
                for m in range(MC):
                    slot = (i * MC + m) % NPRE
                    ps = ppp.tile([128, BLK * 32], F32, tag=f"pre{slot}",
                                  name=f"pre{i}_{m}", padded_shape=[128, 512])
                    for k in range(KC):
                        nc.tensor.matmul(
                            ps[:, :],
                            wpre_sb[k][:, m * 128:(m + 1) * 128],
                            xT_sb[k][:, i * BLK * 32:(i + 1) * BLK * 32],
                            start=(k == 0),
                            stop=(k == KC - 1),
                        )
                    dst = ring3[:, s0:s0 + BLK, m * 32:(m + 1) * 32]
                    src = ps[:, :].rearrange("p (s c) -> p s c", c=32)
                    nc.vector.tensor_scalar(dst, src, bias_sb[:, m:m + 1],
                                            None, OP.add)

            def scan_step(t):
                base = (t % RING) * 512
                slot = t % 2
                ps_r = psp.tile([128, 256], F32, tag=f"pr{slot}", name=f"psr{t}",
                                padded_shape=[128, 512])
                ps_n = psp.tile([128, 256], F32, tag=f"pn{slot}", name=f"psn{t}",
                                padded_shape=[128, 512])
                # Initialize PSUM via identity matmuls:
                # ps_r = xr_t (+r-biases, folded on host into the ring),
                # ps_n = U_b.
                nc.tensor.matmul(ps_r[:, :], id_sb[:, :], ring[:, base:base + 256],
                                 start=True, stop=False, skip_group_check=True)
                nc.tensor.matmul(ps_n[:, :], id_sb[:, :], ub_sb[:, :],
                                 start=True, stop=False, skip_group_check=True)
                for m in range(MC):
                    half = ps_r if m < 8 else ps_n
                    col = (m % 8) * 32
                    for k in range(KC):
                        nc.tensor.matmul(
                            half[:, col:col + 32],
                            uu_sb[k][:, m * 128:(m + 1) * 128],
                            hbf[:, k * 32:(k + 1) * 32],
                            start=False,
                            stop=(k == KC - 1),
                            skip_group_check=True,
                        )
                rt = ew.tile([128, 256], F32, tag="rt", name=f"rt{t}")
                nc.scalar.activation(rt[:, :], ps_r[:, :], AF.Sigmoid)
                n2 = ew.tile([128, 256], F32, tag="n2", name=f"n2{t}")
                nc.vector.tensor_tensor(n2[:, :], rt[:, :], ps_n[:, :], OP.mult)
                an = ew.tile([128, 256], F32, tag="an", name=f"an{t}")
                nc.vector.tensor_tensor(an[:, :], n2[:, :],
                                        ring[:, base + 256:base + 512], OP.add)
                nt = ew.tile([128, 256], F32, tag="nt", name=f"nt{t}")
                nc.scalar.activation(nt[:, :], an[:, :], AF.Tanh)
                d = ew.tile([128, 256], F32, tag="d", name=f"d{t}")
                nc.vector.tensor_tensor(d[:, :], hT[:, :], nt[:, :], OP.subtract)
                e = ew.tile([128, 256], F32, tag="e", name=f"e{t}")
                nc.vector.tensor_tensor(e[:, :], d[:, :], gt_sb[:, :], OP.mult)
                nc.vector.tensor_tensor(hT[:, :], nt[:, :], e[:, :], OP.add)
                nc.vector.tensor_copy(hbf[:, :], hT[:, :])

            precompute_block(0)
            precompute_block(1)
            for i in range(NBLK):
                for u in range(BLK):
                    scan_step(i * BLK + u)
                if i + 2 < NBLK:
                    precompute_block(i + 2)

            nc.sync.dma_start(out=out[:, :], in_=hT[:, :])

    nc.finalize()
    return nc


def _prep_inputs(x, h0, gt, Wr_w, Wr_b, Ur_w, Ur_b, W_w, W_b, U_w, U_b):
    bf = ml_dtypes.bfloat16
    wpreT = np.ascontiguousarray(
        np.concatenate([Wr_w.T, W_w.T], axis=1)).astype(bf)          # [H, 2048]
    uuT = np.ascontiguousarray(
        np.concatenate([Ur_w.T, U_w.T], axis=1)).astype(bf)          # [H, 2048]
    # biasp[:, m] = per-partition bias for precompute chunk m, added during
    # the psum->ring copy: r-chunks get Wr_b+Ur_b, n-chunks get W_b.
    # ubT = U_b broadcast (folded into ps_n by an identity matmul).
    def hmajor_bcast(v):
        return np.ascontiguousarray(
            np.broadcast_to(v.reshape(8, 128).T[:, :, None],
                            (128, 8, 32)).reshape(128, 256))
    bias_cat = np.concatenate([(Wr_b + Ur_b), W_b]).astype(np.float32)
    biasp = np.ascontiguousarray(bias_cat.reshape(MC, 128).T)
    ubT = hmajor_bcast(U_b.astype(np.float32)).astype(bf)
    ident = np.eye(128, dtype=bf)

    in_maps = []
    for c in range(NCORES):
        sl = slice(c * BL, (c + 1) * BL)
        x_loc = x[sl]                                  # [32, 128, 1024]
        xT = np.ascontiguousarray(
            x_loc.transpose(2, 1, 0).reshape(H, BT)).astype(bf)
        h0T = np.ascontiguousarray(
            h0[sl].reshape(BL, 8, 128).transpose(2, 1, 0).reshape(128, 256)
        ).astype(np.float32)
        gtT = np.ascontiguousarray(
            np.broadcast_to(gt[sl].reshape(BL)[None, None, :],
                            (128, 8, 32)).reshape(128, 256)).astype(np.float32)
        in_maps.append({
            "xT": xT, "wpreT": wpreT, "uuT": uuT, "biasp": biasp,
            "ubT": ubT, "gtT": gtT, "h0T": h0T, "ident": ident,
        })
    return in_maps


def kernel(x, h0, gt, Wr_w, Wr_b, Ur_w, Ur_b, Wz_w, Wz_b, Uz_w, Uz_b,
           W_w, W_b, U_w, U_b, _trace=False, _tmpdir=None):
    x = np.asarray(x, np.float32)
    h0 = np.asarray(h0, np.float32)
    gt = np.asarray(gt, np.float32)
    in_maps = _prep_inputs(x, h0, gt,
                           np.asarray(Wr_w, np.float32), np.asarray(Wr_b, np.float32),
                           np.asarray(Ur_w, np.float32), np.asarray(Ur_b, np.float32),
                           np.asarray(W_w, np.float32), np.asarray(W_b, np.float32),
                           np.asarray(U_w, np.float32), np.asarray(U_b, np.float32))
    if "nc" not in _CACHE:
        _CACHE["nc"] = _build_bass()
    res = run_bass_kernel_spmd(_CACHE["nc"], in_maps, core_ids=list(range(NCORES)),
                               trace=_trace, tmpdir=_tmpdir)
    outs = []
    for c in range(NCORES):
        o = np.asarray(res.results[c]["out"], np.float32)       # [128, 256]
        outs.append(o.reshape(128, 8, BL).transpose(2, 1, 0).reshape(BL, H))
    full = np.concatenate(outs, axis=0)                          # [256, 1024]
    if _trace:
        return full, res
    return full
